# revision 2
# baseline (speedup 1.0000x reference)
"""Grouped SwiGLU FFN (8 experts) — expert-parallel Bass kernel for 8 trn2 cores.

Per core (one expert): out = (silu(x@w1) * (x@w3T)) @ w2T.
  x: [T=1024, D=2048], w1: [D, H=4096], w3: [H, D], w2: [D, H].

All matmul operands are float16 — measured fastest PE mode on trn2
(216 ns per 512-col matmul vs 226.5 fp32r, 259 bf16), psum stays fp32;
rel err ~5e-4. Zero on-device transposes — layouts pre-packed on host:
  phase1: g^T[h, t]  = silu(w1^T-tile.T @ x^T) * (w3-tile.T @ x^T)  (per h-tile)
  phase2: out^T[d,t] = sum_h w2-tile.T @ g^T                        (w2 stationary)
v2 structure: phase1 runs all 32 h-tiles into one full-size g buffer
(64KB/partition fp16); phase2 then streams 16 d-tiles, each as two
32-deep psum accumulation groups drained via a small fp32 stage tile
straight to DRAM — no SBUF out accumulator and no vector adds, so the
output DMA finishes ~2us after the last matmul. w1/w3 are packed into
one DRAM tensor (one DMA per h-tile) and the head issues x early so the
first real matmuls start as soon as the DMA ring spins up; a short
dummy-matmul warmup keeps the PE clock ramping until then.
"""

import sys

sys.path.insert(0, "/opt/trn_rl_repo")

import numpy as np

import concourse.bass as bass
from concourse import bacc
import concourse.mybir as mybir
import concourse.tile as tile
from concourse.bass_utils import run_bass_kernel_spmd

E, T, D, H = 8, 1024, 2048, 4096
P = 128
NT = 512            # matmul moving free dim (fp32 psum max)
DT = D // P         # 16 contraction tiles over D
HT = H // P         # 32 h-tiles
TH = T // NT        # 2 t-halves
DTT = D // P        # 16 out^T row tiles
F32 = mybir.dt.float32
F16 = mybir.dt.float16

_CACHE: dict = {}


def _build_nc():
    nc = bacc.Bacc("TRN2", target_bir_lowering=False, debug=False)
    xp = nc.dram_tensor("xp", [DT, P, T], F16, kind="ExternalInput")
    # w1/w3 packed: wp[ht, p, 0, dt, j] = w1[dt*128+p, ht*128+j]
    #               wp[ht, p, 1, dt, j] = w3[ht*128+j, dt*128+p]
    wp = nc.dram_tensor("wp", [HT, P, 2, DT, P], F16, kind="ExternalInput")
    # w2p[dtt, p, ht, j] = w2[dtt*128+j, ht*128+p]
    w2p = nc.dram_tensor("w2p", [DTT, P, HT, P], F16, kind="ExternalInput")
    outT = nc.dram_tensor("outT", [D, T], F32, kind="ExternalOutput")

    with tile.TileContext(nc) as tc:
        with (
            tc.tile_pool(name="xpool", bufs=1) as xpool,
            tc.tile_pool(name="gpool", bufs=1) as gpool,
            tc.tile_pool(name="wpool", bufs=3) as wpool,
            tc.tile_pool(name="w2pool", bufs=2) as w2pool,
            tc.tile_pool(name="spool", bufs=2) as spool,
            tc.tile_pool(name="pspool", bufs=8, space="PSUM") as pspool,
        ):
            def load_w(ht):
                wsb = wpool.tile([P, 2, DT, P], F16, tag="w", name=f"wsb_{ht}")
                nc.sync.dma_start(wsb, wp[ht])
                return wsb

            # PE pstate warmup: dummy accumulation groups on engine-memset
            # tiles run during the ~9us DMA spin-up so the first real
            # matmuls start with the clock already ramping. Three short
            # groups give a fine-grained handoff to real work.
            wrm_w = spool.tile([P, P], F16, tag="wrmw", bufs=1)
            wrm_x = spool.tile([P, NT], F16, tag="wrmx", bufs=1)
            nc.vector.memset(wrm_w, 1.0)
            nc.vector.memset(wrm_x, 1.0)
            for n in (3, 2, 2):
                psw = pspool.tile([P, NT], F32, tag="ps", bufs=4, name="psw")
                for i in range(n):
                    nc.tensor.matmul(
                        psw, lhsT=wrm_w, rhs=wrm_x,
                        start=(i == 0), stop=(i == n - 1),
                    )

            # head: COARSE transfers — each dma_start costs ~0.6us of DGE
            # issue time, so fewer+bigger beats fine-grained splitting.
            # Order feeds the warm-start: first h-tile weights, then the
            # x chunks it consumes, interleaved with the next weight tiles.
            xsb = xpool.tile([P, DT, T], F16, tag="x")

            def load_x(dts):
                for dt_i in dts:
                    nc.sync.dma_start(xsb[:, dt_i, :], xp[dt_i])

            w_stash = {0: load_w(0)}
            load_x(range(4))
            w_stash[1] = load_w(1)
            load_x(range(4, 8))
            w_stash[2] = load_w(2)
            load_x(range(8, DT))

            g = gpool.tile([P, HT, T], F16, tag="g")

            def mm_burst(ps, wv, th, dts):
                ts = slice(th * NT, (th + 1) * NT)
                for dt_i in dts:
                    nc.tensor.matmul(
                        ps,
                        lhsT=wv[:, dt_i],
                        rhs=xsb[:, dt_i, ts],
                        start=(dt_i == 0),
                        stop=(dt_i == DT - 1),
                    )

            def mm_burst2(ps1, ps3, wsb, th, dts):
                # ps1/ps3 interleaved per dt: halves the per-x-chunk PE
                # consumption rate at the head so it matches DMA arrival
                ts = slice(th * NT, (th + 1) * NT)
                for dt_i in dts:
                    nc.tensor.matmul(
                        ps1, lhsT=wsb[:, 0, dt_i], rhs=xsb[:, dt_i, ts],
                        start=(dt_i == 0), stop=(dt_i == DT - 1),
                    )
                    nc.tensor.matmul(
                        ps3, lhsT=wsb[:, 1, dt_i], rhs=xsb[:, dt_i, ts],
                        start=(dt_i == 0), stop=(dt_i == DT - 1),
                    )

            def epilogue(ps1, ps3, ht, th):
                ts = slice(th * NT, (th + 1) * NT)
                sil = spool.tile([P, NT], F32, tag="sil")
                nc.scalar.activation(
                    sil, ps1, mybir.ActivationFunctionType.Silu
                )
                nc.vector.tensor_mul(out=g[:, ht, ts], in0=sil, in1=ps3)

            # warm start: split the first two h-tiles' accumulation into
            # dt halves so 8 psum groups are in the PE queue while the x
            # chunks stream in
            warm = {}
            for ht in range(2):
                wsb = w_stash[ht]
                for th in range(TH):
                    ps1 = pspool.tile([P, NT], F32, tag="ps", bufs=4, name="ps1")
                    ps3 = pspool.tile([P, NT], F32, tag="ps", bufs=4, name="ps3")
                    warm[ht, th] = (ps1, ps3, wsb)
                    mm_burst2(ps1, ps3, wsb, th, range(DT // 2))
            for ht in range(2):
                for th in range(TH):
                    ps1, ps3, wsb = warm[ht, th]
                    mm_burst(ps1, wsb[:, 0], th, range(DT // 2, DT))
                    mm_burst(ps3, wsb[:, 1], th, range(DT // 2, DT))
                    epilogue(ps1, ps3, ht, th)

            # phase 1 steady state
            for ht in range(2, HT):
                wsb = w_stash[ht] if ht in w_stash else load_w(ht)
                for th in range(TH):
                    ps1 = pspool.tile([P, NT], F32, tag="ps", bufs=4, name="ps1")
                    ps3 = pspool.tile([P, NT], F32, tag="ps", bufs=4, name="ps3")
                    mm_burst(ps1, wsb[:, 0], th, range(DT))
                    mm_burst(ps3, wsb[:, 1], th, range(DT))
                    epilogue(ps1, ps3, ht, th)

            # phase 2: per d-tile, one 32-deep accumulation group per
            # t-half, drained through a small fp32 stage tile to DRAM
            for dtt in range(DTT):
                w2sb = w2pool.tile([P, HT, P], F16, tag="w2")
                nc.sync.dma_start(w2sb, w2p[dtt])
                for th in range(TH):
                    ts = slice(th * NT, (th + 1) * NT)
                    po = pspool.tile([P, NT], F32, tag="po", bufs=4, name="po")
                    for ht in range(HT):
                        nc.tensor.matmul(
                            po,
                            lhsT=w2sb[:, ht],
                            rhs=g[:, ht, ts],
                            start=(ht == 0),
                            stop=(ht == HT - 1),
                        )
                    st = spool.tile([P, NT], F32, tag="st", bufs=3)
                    nc.vector.tensor_copy(out=st, in_=po)
                    nc.sync.dma_start(
                        outT[dtt * P : (dtt + 1) * P, ts], st
                    )
    nc.compile()
    return nc


def _pack_inputs(x, w1, w2, w3):
    """Per-expert host-side packing into DMA-linear layouts."""
    in_maps = []
    for e in range(E):
        xe = np.asarray(x[e], dtype=np.float32).astype(np.float16)
        w1e = np.asarray(w1[e], dtype=np.float32).astype(np.float16)
        w2e = np.asarray(w2[e], dtype=np.float32).astype(np.float16)
        w3e = np.asarray(w3[e], dtype=np.float32).astype(np.float16)
        # xp[dt, p, t] = x[t, dt*128+p]
        xp = np.ascontiguousarray(xe.reshape(T, DT, P).transpose(1, 2, 0))
        # w1 part: [ht, p, dt, j] = w1[dt*128+p, ht*128+j]
        w1p = w1e.reshape(DT, P, HT, P).transpose(2, 1, 0, 3)
        # w3 part: [ht, p, dt, j] = w3[ht*128+j, dt*128+p]
        w3p = w3e.reshape(HT, P, DT, P).transpose(0, 3, 2, 1)
        # packed: wp[ht, p, 2, dt, j]
        wpk = np.ascontiguousarray(np.stack([w1p, w3p], axis=2))
        # w2p[dtt, p, ht, j] = w2[dtt*128+j, ht*128+p]
        w2p = np.ascontiguousarray(
            w2e.reshape(DTT, P, HT, P).transpose(0, 3, 2, 1)
        )
        in_maps.append({"xp": xp, "wp": wpk, "w2p": w2p})
    return in_maps


def kernel(x, w1, w2, w3, _trace=False, _trace_kwargs=None):
    if "nc" not in _CACHE:
        _CACHE["nc"] = _build_nc()
    nc = _CACHE["nc"]
    in_maps = _pack_inputs(x, w1, w2, w3)
    kw = {}
    if _trace:
        kw = {"trace": True}
        if _trace_kwargs:
            kw.update(_trace_kwargs)
    res = run_bass_kernel_spmd(nc, in_maps, core_ids=list(range(E)), **kw)
    out = np.empty((E, T, D), dtype=np.float32)
    for e in range(E):
        out[e] = res.results[e]["outT"].T
    if _trace:
        _CACHE["last_results"] = res
    return out
